# revision 24
# baseline (speedup 1.0000x reference)
"""Toeplitz bias kernel for trn2 (8 NeuronCores).

bias[h, j, i] = exp(w_[h] - offset[h])[2*L-2 + j - i]   with L = 2048.

Let q[k] = exp(w_rev[h] - offset[h])[k] where w_rev is w_ reversed along
the table axis (host-side layout transform). Then

  bias[h, j, i] = q[(L-1) - j + i]

Device computes devout[h, jj, i] = q[jj + i] (the row-flipped bias:
jj = L-1-j), which only needs NON-negative offsets everywhere:

  - win[h] = w_rev[h] - offset[h] is packed on host as fp16 (the exp
    upconverts to f32; ~1.6e-3 rel err, well under the 2e-2 gate, and
    it halves the staircase load traffic).
  - per head the staircase tile is built directly from DRAM with an
    overlapping access pattern: wrep[t, c] = win[h, t + c]
    (AP = [(1, 128), (1, COLS)], COLS = 15*128 + L = 3968; max index
    127 + 3967 = 4094 = S-1, always in bounds). The load is split into
    two column chunks so exp can start before the full tile lands.
  - exp runs on the activation engine per (head, chunk): qr = exp(wrep),
    fp16 in -> f32 out, all 128 partitions in parallel. The block-0
    store is issued between chunk A and chunk B so stores start early.
  - every 128-row output block is a plain slice store:
      devout[h, j0:j0+128, :] = qr[:, j0:j0+L]      (1MB DMA per block)

Host flips the row axis when assembling the full output:
  bias[h, j, :] = devout[h, L-1-j, :].

Heads are sharded 2 per core across the 8 cores; loads/stores of head 0
ride the sync (SP) hardware DMA ring and head 1 the scalar (Activation)
ring, so the two hardware queues split the 32MB of stores evenly and
all 16 DMA engines stay saturated (~420 GB/s, the per-core fabric
ceiling) for the whole store phase.

Measured: ~97-102us HW exec (baseline staircase-by-doubling kernel:
138.7us). Floor is ~93us: 32MB of stores + 2MB of loads at 424 GB/s
plus ~6.5us framework preamble and ~5us drain/teardown.

Failed variants (for the record): packing 2 output rows per partition
(64-partition staircase, 3-dim store APs) halves load bytes but the two
in-flight descriptors per partition conflict on the SBUF partition port
and store packets run at half rate (182us). Splitting chunk-A loads
across both rings adds no overlap (the preamble, not the load, gates
the first exp). Splitting block 0 into column-half stores gated on
1024-col exps fills the ramp but loses it back to 4KB-packet overhead
(packets below 8KB pay a fixed cost; >=8KB run at the ceiling). Merged
3-dim "mega stores" with a block-major outer dim are rejected by the
walrus lowering (SBUF AP dim0 must be the partition dim). A minimal
2-DMA kernel measures 13us: ~5us launch pipeline (static-DMA cold
start + per-engine ucode loads + barriers) and a ~7us end-of-kernel
semaphore-reset storm that a real kernel hides under its DMA phase.
Shrinking DMAQueue.num_queues from 16 to 15 drops the queue to 8
engines (power-of-two mapping), halving bandwidth (180us). Each
dma_start costs ~0.7-1.3us of issuing-engine dispatch time, which with
the ~6.8us preamble and ~1.5us completion-semaphore latency pins the
earliest possible first store at ~12.5us — the current schedule is
within ~1.5us of that bound. Sub-splitting the gating chunk-A load
(0.5MB -> 2x0.25MB) does not advance the first exp: head-of-pipeline
load completion is dominated by queue-start latency + burst drain, not
chunk size (measured identical 11.8us exp start, chain shifted ~1us
later by the extra dependency hop).
"""

import numpy as np

H = 16
L = 2048
S = 2 * L - 1  # 4095
N_CORES = 8
HPC = H // N_CORES  # heads per core
NBLK = L // 128  # 16 row blocks per head
COLS = (NBLK - 1) * 128 + L  # 3968 staircase columns

_cached_nc = None


def _build_nc():
    import concourse.bacc as bacc
    import concourse.mybir as mybir
    import concourse.tile as tile
    from concourse.ap import AP

    nc = bacc.Bacc("TRN2", target_bir_lowering=False)
    f32 = mybir.dt.float32
    f16 = mybir.dt.float16
    win = nc.dram_tensor("win", [HPC, S], f16, kind="ExternalInput")
    out = nc.dram_tensor("out", [HPC, L, L], f32, kind="ExternalOutput")

    # column split: chunk A covers the first store block, B the rest
    CA = L  # 2048
    CB = COLS - CA  # 1920

    with tile.TileContext(nc) as tc:
        with tc.tile_pool(name="p", bufs=1) as pool:
            rings = (nc.sync, nc.scalar)
            wreps, qrs = [], []
            # loads first (both rings pull from HBM immediately); split
            # per head into two column chunks so exp can start before the
            # full staircase has landed
            for h in range(HPC):
                wrep = pool.tile([128, COLS], f16, tag=f"wrep{h}")
                wreps.append(wrep)
                rings[h % 2].dma_start(
                    wrep[:, 0:CA], AP(win, h * S, [(1, 128), (1, CA)])
                )
            for h in range(HPC):
                rings[h % 2].dma_start(
                    wreps[h][:, CA:COLS],
                    AP(win, h * S + CA, [(1, 128), (1, CB)]),
                )
            for h in range(HPC):
                qr = pool.tile([128, COLS], f32, tag=f"qr{h}")
                qrs.append(qr)
            # exp chunk A per head, then first store, then exp chunk B
            for h in range(HPC):
                nc.scalar.activation(
                    qrs[h][:, 0:CA],
                    wreps[h][:, 0:CA],
                    mybir.ActivationFunctionType.Exp,
                )
            for h in range(HPC):
                rings[h % 2].dma_start(out[h, 0:128, :], qrs[h][:, 0:L])
            for h in range(HPC):
                nc.scalar.activation(
                    qrs[h][:, CA:COLS],
                    wreps[h][:, CA:COLS],
                    mybir.ActivationFunctionType.Exp,
                )
            for b in range(1, NBLK):
                j0 = 128 * b
                for h in range(HPC):
                    rings[h % 2].dma_start(
                        out[h, j0 : j0 + 128, :], qrs[h][:, j0 : j0 + L]
                    )
    nc.compile()
    return nc


def _get_nc():
    global _cached_nc
    if _cached_nc is None:
        _cached_nc = _build_nc()
    return _cached_nc


def _make_in_maps(w_, offset):
    w_ = np.asarray(w_, dtype=np.float32)
    offset = np.asarray(offset, dtype=np.float32)
    win = (w_ - offset[:, None])[:, ::-1].astype(np.float16)  # [H, S]
    in_maps = []
    for c in range(N_CORES):
        sl = slice(c * HPC, (c + 1) * HPC)
        in_maps.append({"win": np.ascontiguousarray(win[sl])})
    return in_maps


def run(w_, offset, trace=False, **trace_kw):
    import concourse.bass_utils as bu
    from concourse.bass_utils import run_bass_kernel_spmd

    if trace:
        # no fish bucket in this container; keep artifacts local
        bu.upload_artifacts = lambda tmpdir: "local://" + str(tmpdir)

    nc = _get_nc()
    in_maps = _make_in_maps(w_, offset)
    try:
        res = run_bass_kernel_spmd(
            nc, in_maps, list(range(N_CORES)), trace=trace, **trace_kw
        )
    except Exception:
        # Transient device errors (NRT_EXEC_UNIT_UNRECOVERABLE) have been
        # observed on first runs and clear on retry. Best-effort backend
        # reset, then one retry; if that also fails, propagate.
        try:
            import jax

            jax.clear_backends()
        except Exception:
            pass
        res = run_bass_kernel_spmd(
            nc, in_maps, list(range(N_CORES)), trace=trace, **trace_kw
        )
    # device wrote the row-flipped bias; undo the flip while gathering
    parts = [np.asarray(r["out"])[:, ::-1, :] for r in res.results]
    full = np.concatenate(parts, axis=0)  # [H, L, L]
    return full, res


def kernel(w_, offset, seq_len=None, **_ignored):
    full, _ = run(w_, offset, trace=False)
    return full
